# revision 5
# baseline (speedup 1.0000x reference)
"""Trainium2 Bass kernel for nn_Comm_OUT (Linear+BN+PReLU -> 20-step GRU ->
BN+PReLU -> 1x1 conv -> BN+PReLU -> linear head), data-parallel over 8 cores.

Layout strategy: everything on-chip is kept "transposed" (channels on SBUF
partitions, batch on the free dim) so the GRU recurrence never needs an
on-chip transpose:
    gh.T[3H, B] = W_hh @ h.T   (W_hh.T tiles are the stationary operand)
The recurrence matmuls run in fp8-e4m3 DoubleRow mode (K=256 per
instruction), per-step `gi + gh` adds are folded into the PE via identity-
matmul PSUM preloads, biases fold into per-partition activation operands,
and BN+PReLU collapses into single Prelu activations. The r/z sigmoids are
fused into one ACT instruction via a two-bank access pattern. The stage-2
prelu tail (w2b/max) runs on the otherwise idle GPSIMD engine.
"""

import numpy as np
import ml_dtypes

import concourse.bacc as bacc
import concourse.mybir as mybir
import concourse.tile as tile
from concourse import bass_utils

AF = mybir.ActivationFunctionType
OP = mybir.AluOpType
F32 = mybir.dt.float32
F32R = mybir.dt.float32r
BF16 = mybir.dt.bfloat16
FP8 = mybir.dt.float8e4
DR = mybir.MatmulPerfMode.DoubleRow

E, S, F, H, C, L = 64, 128, 640, 256, 32, 20
EPS = 1e-5
NCORES = 8
B = E * S              # 8192
BC = B // NCORES       # 1024 batch rows per core
NCH = 2                # chunks per core
CB = BC // NCH         # 512 batch rows per chunk (PSUM-bank friendly)

# const-vector column indices (packed [128, NV] tensor, one column per
# per-partition operand vector)
CV_S1, CV_T1, CV_GIB, CV_BHN, CV_S2, CV_T2, CV_S3, CV_T3 = 0, 2, 4, 10, 12, 14, 16, 18
CV_BMU, CV_A1, CV_A2, CV_A3 = 20, 21, 22, 23
NV = 24

# bf16 weight-pack column offsets (whh lives in a separate fp8 tensor)
WP_LIN, WP_IH, WP_C, WP_MU, WP_ID = 0, 1280, 2816, 3328, 3392
WP_COLS = 3520

_CACHE: dict = {}


def _mm(x):
    if x.dtype == F32:
        return x.bitcast(F32R)
    return x


def build_program(bhn_zero=True):
    BHN_ZERO = bhn_zero
    nc = bacc.Bacc("TRN2", target_bir_lowering=False, debug=False)

    xT_h = nc.dram_tensor("xT", [128, 5 * BC], BF16, kind="ExternalInput")
    wp_h = nc.dram_tensor("wp", [128, WP_COLS], BF16, kind="ExternalInput")
    w8_h = nc.dram_tensor("w8", [128, 1536], FP8, kind="ExternalInput")
    cv_h = nc.dram_tensor("cv", [128, NV], F32, kind="ExternalInput")
    aux_h = nc.dram_tensor("aux", [1, H + CB], BF16, kind="ExternalInput")
    out_h = nc.dram_tensor("out", [BC, L * C], F32, kind="ExternalOutput")

    with tile.TileContext(nc) as tc:
        with (
            tc.tile_pool(name="consts", bufs=1) as cpool,
            tc.tile_pool(name="gi", bufs=1) as gip,
            tc.tile_pool(name="hp", bufs=2) as hp,
            tc.tile_pool(name="hp8", bufs=2) as hp8,
            tc.tile_pool(name="gates", bufs=3) as gp,
            tc.tile_pool(name="s24", bufs=2) as sp,
            tc.tile_pool(name="ps2", bufs=3, space="PSUM") as ps2,
            tc.tile_pool(name="ps1", bufs=2, space="PSUM") as ps1,
        ):
            cvt = cpool.tile([128, NV], F32, tag="cv")
            nc.sync.dma_start(cvt[:], cv_h[:])
            wpt = cpool.tile([128, WP_COLS], BF16, tag="wpt")
            nc.sync.dma_start(wpt[:], wp_h[:])
            w8t = cpool.tile([128, 1536], FP8, tag="w8t")
            nc.sync.dma_start(w8t[:], w8_h[:])
            aux = cpool.tile([1, H + CB], BF16, tag="aux")
            nc.sync.dma_start(aux[:], aux_h[:])
            wlin_t = wpt[:, WP_LIN:WP_LIN + 1280]
            wih_t = wpt[:, WP_IH:WP_IH + 1536]
            wc_t = wpt[:, WP_C:WP_C + 512]
            wmu_t = wpt[:, WP_MU:WP_MU + 64]
            idtb = wpt[:, WP_ID:WP_ID + 128]

            def whh8(g, m):  # fp8 DR stationary [128, 2, 128] for gate g half m
                base = (g * 2 + m) * 256
                return w8t[:, base:base + 256].rearrange("p (k m) -> p k m", k=2)

            def pp(col):  # per-partition operand column
                return cvt[:, col : col + 1]

            # ---- stage 1: x1 = prelu(bn(x @ W_lin.T)), gi = x1 @ W_ih.T ----
            gi_tiles = []
            with tc.tile_pool(name="stage1", bufs=1) as xp:
                xt = xp.tile([128, 5 * BC], BF16, tag="xT")
                nc.sync.dma_start(xt[:], xT_h[:])
                for c in range(NCH):
                    ps_x1 = ps2.tile([128, 2 * CB], F32, tag="g")
                    for k in range(5):
                        for m in range(2):
                            nc.tensor.matmul(
                                ps_x1[:, m * CB : (m + 1) * CB],
                                wlin_t[:, k * H + m * 128 : k * H + (m + 1) * 128],
                                xt[:, k * BC + c * CB : k * BC + (c + 1) * CB],
                                start=(k == 0),
                                stop=(k == 4),
                            )
                    x1p = xp.tile([128, 2 * CB], BF16, tag=f"x1p{c}")
                    for m in range(2):
                        nc.scalar.activation(
                            x1p[:, m * CB : (m + 1) * CB],
                            ps_x1[:, m * CB : (m + 1) * CB],
                            AF.Prelu,
                            bias=pp(CV_T1 + m),
                            scale=pp(CV_S1 + m),
                            alpha=pp(CV_A1),
                        )
                    gi = gip.tile([128, 6 * CB], BF16, tag=f"gi{c}")
                    gi_tiles.append(gi)
                    for m in range(6):
                        ps_gi = ps1.tile([128, CB], F32, tag="p")
                        for k in range(2):
                            nc.tensor.matmul(
                                ps_gi[:],
                                wih_t[:, k * 3 * H + m * 128 : k * 3 * H + (m + 1) * 128],
                                x1p[:, k * CB : (k + 1) * CB],
                                start=(k == 0),
                                stop=(k == 1),
                            )
                        # gi' = gi + (b_ih [+ b_hh for r,z]) folded via bias
                        nc.scalar.activation(
                            gi[:, m * CB : (m + 1) * CB],
                            ps_gi[:],
                            AF.Identity,
                            bias=pp(CV_GIB + m),
                        )

            # ---- GRU + stages 2..4; chunks interleaved per step so their
            # independent dependency chains overlap across engines ----
            hs = []
            h8s = []
            packs = [None] * NCH
            gin_bf = [gi_tiles[c][:, 4 * CB : 6 * CB] for c in range(NCH)]
            for c in range(NCH):
                h = hp.tile([128, 2 * CB], BF16, tag=f"h{c}")
                nc.vector.memset(h[:], 0.0)
                hs.append(h)
                h8 = hp8.tile([128, 2 * CB], FP8, tag=f"h8{c}")
                nc.vector.memset(h8[:], 0.0)
                h8s.append(h8)
            w2s = [None] * NCH
            y2s = [None] * NCH

            def emit_conv(t, c):
                """conv matmuls for step t (emitted at period t+1: inputs ready)"""
                ps_cv = ps2.tile([128, 2 * CB], F32, tag="g", name=f"pscv{c}")
                w2 = w2s[c]
                for m in range(2):
                    for k in range(2):
                        nc.tensor.matmul(
                            ps_cv[:, m * CB : (m + 1) * CB],
                            wc_t[:, k * H + m * 128 : k * H + (m + 1) * 128],
                            w2[:, k * CB : (k + 1) * CB],
                            start=(k == 0),
                            stop=False,
                        )
                    nc.tensor.matmul(
                        ps_cv[:, m * CB : (m + 1) * CB],
                        aux[:, m * 128 : (m + 1) * 128],
                        aux[:, H : H + CB],
                        start=False,
                        stop=True,
                    )
                return ps_cv

            def emit_prelu3(ps_cv, c):
                y2 = gp.tile([128, 2 * CB], BF16, tag="y2", name=f"y2_{c}")
                nc.scalar.activation(y2[:], ps_cv[:], AF.Prelu, alpha=pp(CV_A3))
                y2s[c] = y2

            def emit_wmu(t, c):
                """W_mu head for step t (emitted at period t+2), col-tiled so 4
                consecutive steps pack one PSUM bank [4l x 32c, b]"""
                j = t % 4
                if j == 0:
                    packs[c] = ps1.tile([128, CB], F32, tag="p", name=f"pack{c}")
                ps_pack = packs[c]
                y2 = y2s[c]
                for k in range(2):
                    nc.tensor.matmul(
                        ps_pack[32 * j : 32 * (j + 1), :],
                        wmu_t[:, k * C : (k + 1) * C],
                        y2[:, k * CB : (k + 1) * CB],
                        start=(k == 0),
                        stop=(k == 1),
                        tile_position=(0, 32 * j),
                    )

            def emit_pack_out(t, c):
                if t % 4 != 3:
                    return
                ps_pack = packs[c]
                pk = sp.tile([128, CB], F32, tag="pk", name=f"pk{c}")
                nc.scalar.activation(pk[:], ps_pack[:], AF.Identity, bias=pp(CV_BMU))
                tr = sp.tile([128, CB], F32, tag="tr", name=f"tr{c}")
                nc.vector.transpose(tr[:], pk[:])
                l4 = t // 4
                dst = out_h[:].rearrange(
                    "(c bh bl) (l4 li cc) -> c l4 li bl bh cc",
                    c=NCH, bh=CB // 32, bl=32, l4=L // 4, li=4, cc=C,
                )
                for li in range(4):
                    nc.sync.dma_start(
                        dst[c, l4, li],
                        tr[32 * li : 32 * (li + 1), :].rearrange(
                            "p (bh cc) -> p bh cc", bh=CB // 32
                        ),
                    )

            def emit_gate_mms_chunk(c, psr, psz, psn):
                # identity preloads put gi_r/gi_z into PSUM, then one fp8
                # DoubleRow matmul per (gate, half) accumulates the full K=256
                # W_hh contribution
                h8v = h8s[c][:].rearrange("p (k n) -> p k n", k=2)
                for g, ps in ((0, psr), (1, psz)):
                    for m in range(2):
                        nc.tensor.matmul(
                            ps[:, m * CB : (m + 1) * CB],
                            idtb,
                            gi_tiles[c][:, (2 * g + m) * CB : (2 * g + m + 1) * CB],
                            start=True,
                            stop=False,
                        )
                for g, ps in ((0, psr), (1, psz), (2, psn)):
                    for m in range(2):
                        nc.tensor.matmul(
                            ps[:, m * CB : (m + 1) * CB],
                            whh8(g, m),
                            h8v,
                            start=(g == 2),
                            stop=True,
                            perf_mode=DR,
                        )

            for t in range(L):
                psr, psz, psn = [], [], []
                for c in range(NCH):
                    psr.append(ps2.tile([128, 2 * CB], F32, tag="g", name=f"psr{c}"))
                    psz.append(ps2.tile([128, 2 * CB], F32, tag="g", name=f"psz{c}"))
                    psn.append(ps2.tile([128, 2 * CB], F32, tag="g", name=f"psn{c}"))
                for c in range(NCH):
                    emit_gate_mms_chunk(c, psr[c], psz[c], psn[c])
                # ready PE work fills the PE tail while the gate chains run
                pscs = [None] * NCH
                if t >= 1:
                    for c in range(NCH):
                        pscs[c] = emit_conv(t - 1, c)
                if t >= 2:
                    for c in range(NCH):
                        emit_wmu(t - 2, c)
                # per-chunk gate chains (staggered): chunk c's chain runs
                # while chunk c+1's matmuls are still on the PE
                for c in range(NCH):
                    r_sb = gp.tile([128, 2 * CB], BF16, tag="r", name=f"r{c}")
                    nc.scalar.activation(r_sb[:], psr[c][:], AF.Sigmoid)
                    z_sb = gp.tile([128, 2 * CB], BF16, tag="z", name=f"z{c}")
                    nc.scalar.activation(z_sb[:], psz[c][:], AF.Sigmoid)
                    t_sb = gp.tile([128, 2 * CB], BF16, tag="t", name=f"t{c}")
                    if BHN_ZERO:
                        nc.vector.tensor_tensor(t_sb[:], psn[c][:], r_sb[:], OP.mult)
                    else:
                        for m in range(2):
                            nc.vector.scalar_tensor_tensor(
                                t_sb[:, m * CB : (m + 1) * CB],
                                psn[c][:, m * CB : (m + 1) * CB],
                                pp(CV_BHN + m),
                                r_sb[:, m * CB : (m + 1) * CB],
                                op0=OP.add,
                                op1=OP.mult,
                            )
                    a_sb = gp.tile([128, 2 * CB], BF16, tag="a", name=f"a{c}")
                    nc.vector.tensor_tensor(a_sb[:], t_sb[:], gin_bf[c], OP.add)
                    n_sb = gp.tile([128, 2 * CB], BF16, tag="n", name=f"n{c}")
                    nc.scalar.activation(n_sb[:], a_sb[:], AF.Tanh)
                    u_sb = gp.tile([128, 2 * CB], BF16, tag="u", name=f"u{c}")
                    nc.vector.tensor_tensor(u_sb[:], hs[c][:], n_sb[:], OP.subtract)
                    v_sb = gp.tile([128, 2 * CB], BF16, tag="v", name=f"v{c}")
                    nc.vector.tensor_tensor(v_sb[:], z_sb[:], u_sb[:], OP.mult)
                    # h' = n + z*(h - n)
                    h = hp.tile([128, 2 * CB], BF16, tag=f"h{c}", name=f"h{c}")
                    nc.vector.tensor_tensor(h[:], n_sb[:], v_sb[:], OP.add)
                    hs[c] = h
                    if t < L - 1:
                        h8 = hp8.tile([128, 2 * CB], FP8, tag=f"h8{c}", name=f"h8{c}")
                        nc.vector.tensor_copy(h8[:], h[:])
                        h8s[c] = h8
                    # fill ACT while the other chunk's matmuls finish
                    if t >= 1 and pscs[c] is not None:
                        emit_prelu3(pscs[c], c)
                # stage-2 prelu (off the recurrence): affine on DVE (4x TS),
                # alpha-mult + max on the idle GPSIMD engine
                for c in range(NCH):
                    w2a = gp.tile([128, 2 * CB], BF16, tag="w2a", name=f"w2a{c}")
                    for m in range(2):
                        nc.gpsimd.tensor_scalar(
                            w2a[:, m * CB : (m + 1) * CB],
                            hs[c][:, m * CB : (m + 1) * CB],
                            pp(CV_S2 + m),
                            pp(CV_T2 + m),
                            op0=OP.mult,
                            op1=OP.add,
                        )
                    w2b = gp.tile([128, 2 * CB], BF16, tag="w2b", name=f"w2b{c}")
                    nc.gpsimd.tensor_scalar_mul(w2b[:], w2a[:], pp(CV_A2))
                    w2 = gp.tile([128, 2 * CB], BF16, tag="w2", name=f"w2_{c}")
                    # prelu(x) = max(a*x, x) for 0 <= a <= 1
                    nc.vector.tensor_tensor(w2[:], w2a[:], w2b[:], OP.max)
                    w2s[c] = w2
                if t >= 2:
                    for c in range(NCH):
                        emit_pack_out(t - 2, c)
            # epilogue: drain the stage pipeline, chunk-interleaved
            pscs = [emit_conv(L - 1, c) for c in range(NCH)]
            for c in range(NCH):
                emit_wmu(L - 2, c)
            for c in range(NCH):
                emit_prelu3(pscs[c], c)
            for c in range(NCH):
                emit_pack_out(L - 2, c)
            for c in range(NCH):
                emit_wmu(L - 1, c)
            for c in range(NCH):
                emit_pack_out(L - 1, c)

    nc.compile()
    return nc


def _prep_inputs(inputs):
    f32 = np.float32
    x = np.ascontiguousarray(np.asarray(inputs["h_w_action"], f32).reshape(B, F))
    W_lin = np.asarray(inputs["W_lin"], f32)
    b_lin = np.asarray(inputs["b_lin"], f32)
    W_ih = np.asarray(inputs["W_ih"], f32)
    W_hh = np.asarray(inputs["W_hh"], f32)
    b_ih = np.asarray(inputs["b_ih"], f32)
    b_hh = np.asarray(inputs["b_hh"], f32)
    Wc = np.asarray(inputs["Wc"], f32)
    bc = np.asarray(inputs["bc"], f32)
    W_mu = np.asarray(inputs["W_mu"], f32)
    b_mu = np.asarray(inputs["b_mu"], f32)

    def bnfold(g, beta, m, v):
        s = g / np.sqrt(v + EPS)
        return s, beta - m * s

    s1, t1 = bnfold(inputs["g1"], inputs["beta1"], inputs["m1"], inputs["v1"])
    s2, t2 = bnfold(inputs["g2"], inputs["beta2"], inputs["m2"], inputs["v2"])
    s3, t3 = bnfold(inputs["g3"], inputs["beta3"], inputs["m3"], inputs["v3"])
    s1, t1, s2, t2, s3, t3 = (np.asarray(a, f32) for a in (s1, t1, s2, t2, s3, t3))
    t1 = t1 + s1 * b_lin          # fold linear bias into bn1 shift
    t3 = t3 + s3 * bc             # fold conv bias into bn3 shift
    gib = b_ih.copy()
    gib[: 2 * H] += b_hh[: 2 * H]  # fold b_hh into gi for the r,z gates
    bhn = b_hh[2 * H :]

    cv = np.zeros((128, NV), f32)
    for col, vec in ((CV_S1, s1), (CV_T1, t1), (CV_S2, s2), (CV_T2, t2),
                     (CV_S3, s3), (CV_T3, t3)):
        cv[:, col] = vec[:128]
        cv[:, col + 1] = vec[128:]
    for m in range(6):
        cv[:, CV_GIB + m] = gib[m * 128 : (m + 1) * 128]
    cv[:, CV_BHN] = bhn[:128]
    cv[:, CV_BHN + 1] = bhn[128:]
    cv[:, CV_BMU] = np.tile(b_mu, 4)
    cv[:, CV_A1] = f32(np.asarray(inputs["a1"]).reshape(-1)[0])
    cv[:, CV_A2] = f32(np.asarray(inputs["a2"]).reshape(-1)[0])
    cv[:, CV_A3] = f32(np.asarray(inputs["a3"]).reshape(-1)[0])

    bf = ml_dtypes.bfloat16
    f8 = ml_dtypes.float8_e4m3

    def kmaj(a, kt):  # [kt*128, n] -> [128, kt*n] k-tile-major columns
        n = a.shape[1]
        return a.reshape(kt, 128, n).transpose(1, 0, 2).reshape(128, kt * n)

    wp = np.concatenate(
        [
            kmaj(np.ascontiguousarray(W_lin.T), 5),
            kmaj(np.ascontiguousarray(W_ih.T), 2),
            kmaj(np.ascontiguousarray((Wc * s3[:, None]).T), 2),
            kmaj(np.ascontiguousarray(W_mu.T), 2),
            np.eye(128, dtype=f32),
        ],
        axis=1,
    ).astype(bf)
    # fp8 DoubleRow pack: [(gate, half) -> [128, k-tile(2), 128]]
    WhhT = np.ascontiguousarray(W_hh.T)       # [H(k), 3H]
    w8 = np.zeros((128, 1536), f8)
    for g in range(3):
        for m in range(2):
            for k in range(2):
                blk = WhhT[k * 128:(k + 1) * 128,
                           g * H + m * 128: g * H + (m + 1) * 128]
                col = (g * 2 + m) * 256 + k * 128
                w8[:, col:col + 128] = blk.astype(f8)
    shared = {
        "wp": np.ascontiguousarray(wp),
        "w8": np.ascontiguousarray(w8),
        "cv": cv,
        "aux": np.concatenate([t3, np.ones(CB, f32)]).reshape(1, -1).astype(bf),
    }
    in_maps = []
    for i in range(NCORES):
        m = dict(shared)
        xtc = np.ascontiguousarray(x[i * BC : (i + 1) * BC, :].T)  # [640, BC]
        m["xT"] = np.ascontiguousarray(kmaj(xtc, 5).astype(bf))
        in_maps.append(m)
    return in_maps


def kernel(**inputs) -> np.ndarray:
    bhn_zero = bool(np.all(np.asarray(inputs["b_hh"])[2 * H :] == 0))
    key = ("nc", bhn_zero)
    if key not in _CACHE:
        _CACHE[key] = build_program(bhn_zero)
    nc = _CACHE[key]
    _CACHE["last"] = nc
    in_maps = _prep_inputs(inputs)
    res = bass_utils.run_bass_kernel_spmd(nc, in_maps, core_ids=list(range(NCORES)))
    outs = [np.asarray(r["out"], np.float32) for r in res.results]
    return np.concatenate(outs, axis=0).reshape(E, S, L, C)


# revision 7
# speedup vs baseline: 2.1814x; 2.1814x over previous
"""Trainium2 Bass kernel for nn_Comm_OUT (Linear+BN+PReLU -> 20-step GRU ->
BN+PReLU -> 1x1 conv -> BN+PReLU -> linear head), data-parallel over 8 cores.

Layout strategy: everything on-chip is kept "transposed" (channels on SBUF
partitions, batch on the free dim) so the GRU recurrence never needs an
on-chip transpose:
    gh.T[3H, B] = W_hh @ h.T   (W_hh.T tiles are the stationary operand)
The recurrence matmuls run in fp8-e4m3 DoubleRow mode (K=256 per
instruction), per-step `gi + gh` adds are folded into the PE via identity-
matmul PSUM preloads, biases fold into per-partition activation operands,
and BN+PReLU collapses into single Prelu activations. The r/z sigmoids are
fused into one ACT instruction via a two-bank access pattern. The stage-2
prelu tail (w2b/max) runs on the otherwise idle GPSIMD engine.
"""

import numpy as np
import ml_dtypes

import concourse.bacc as bacc
import concourse.mybir as mybir
import concourse.tile as tile
from concourse import bass_utils

AF = mybir.ActivationFunctionType
OP = mybir.AluOpType
F32 = mybir.dt.float32
F32R = mybir.dt.float32r
BF16 = mybir.dt.bfloat16
FP8 = mybir.dt.float8e4
DR = mybir.MatmulPerfMode.DoubleRow

E, S, F, H, C, L = 64, 128, 640, 256, 32, 20
EPS = 1e-5
NCORES = 8
B = E * S              # 8192
BC = B // NCORES       # 1024 batch rows per core
NCH = 2                # chunks per core
CB = BC // NCH         # 512 batch rows per chunk (PSUM-bank friendly)

# const-vector column indices (packed [128, NV] tensor, one column per
# per-partition operand vector)
CV_S1, CV_T1, CV_GIB, CV_BHN, CV_S2, CV_T2, CV_S3, CV_T3 = 0, 2, 4, 10, 12, 14, 16, 18
CV_BMU, CV_A1, CV_A2, CV_A3 = 20, 21, 22, 23
NV = 24

# bf16 weight-pack column offsets (whh lives in a separate fp8 tensor)
WP_LIN, WP_IH, WP_C, WP_MU, WP_ID = 0, 1280, 2816, 3328, 3392
WP_COLS = 3520

_CACHE: dict = {}


def _mm(x):
    if x.dtype == F32:
        return x.bitcast(F32R)
    return x


def build_program(bhn_zero=True):
    BHN_ZERO = bhn_zero
    nc = bacc.Bacc("TRN2", target_bir_lowering=False, debug=False)

    xT_h = nc.dram_tensor("xT", [128, 5 * BC], BF16, kind="ExternalInput")
    wp_h = nc.dram_tensor("wp", [128, WP_COLS], BF16, kind="ExternalInput")
    w8_h = nc.dram_tensor("w8", [128, 1536], FP8, kind="ExternalInput")
    cv_h = nc.dram_tensor("cv", [128, NV], F32, kind="ExternalInput")
    aux_h = nc.dram_tensor("aux", [1, H + CB], BF16, kind="ExternalInput")
    out_h = nc.dram_tensor("out", [BC, L * C], F32, kind="ExternalOutput")

    with tile.TileContext(nc) as tc:
        with (
            tc.tile_pool(name="consts", bufs=1) as cpool,
            tc.tile_pool(name="gi", bufs=1) as gip,
            tc.tile_pool(name="hp", bufs=2) as hp,
            tc.tile_pool(name="hp8", bufs=2) as hp8,
            tc.tile_pool(name="gates", bufs=3) as gp,
            tc.tile_pool(name="s24", bufs=2) as sp,
            tc.tile_pool(name="ps2", bufs=3, space="PSUM") as ps2,
            tc.tile_pool(name="ps1", bufs=2, space="PSUM") as ps1,
        ):
            cvt = cpool.tile([128, NV], F32, tag="cv")
            nc.sync.dma_start(cvt[:], cv_h[:])
            wpt = cpool.tile([128, WP_COLS], BF16, tag="wpt")
            nc.sync.dma_start(wpt[:], wp_h[:])
            w8t = cpool.tile([128, 1536], FP8, tag="w8t")
            nc.sync.dma_start(w8t[:], w8_h[:])
            aux = cpool.tile([1, H + CB], BF16, tag="aux")
            nc.sync.dma_start(aux[:], aux_h[:])
            wlin_t = wpt[:, WP_LIN:WP_LIN + 1280]
            wih_t = wpt[:, WP_IH:WP_IH + 1536]
            wc_t = wpt[:, WP_C:WP_C + 512]
            wmu_t = wpt[:, WP_MU:WP_MU + 64]
            idtb = wpt[:, WP_ID:WP_ID + 128]

            def whh8(g, m):  # fp8 DR stationary [128, 2, 128] for gate g half m
                base = (g * 2 + m) * 256
                return w8t[:, base:base + 256].rearrange("p (k m) -> p k m", k=2)

            def pp(col):  # per-partition operand column
                return cvt[:, col : col + 1]

            # ---- stage 1: x1 = prelu(bn(x @ W_lin.T)), gi = x1 @ W_ih.T ----
            gi_tiles = []
            with tc.tile_pool(name="stage1", bufs=1) as xp:
                xt = xp.tile([128, 5 * BC], BF16, tag="xT")
                nc.sync.dma_start(xt[:], xT_h[:])
                for c in range(NCH):
                    ps_x1 = ps2.tile([128, 2 * CB], F32, tag="g")
                    for k in range(5):
                        for m in range(2):
                            nc.tensor.matmul(
                                ps_x1[:, m * CB : (m + 1) * CB],
                                wlin_t[:, k * H + m * 128 : k * H + (m + 1) * 128],
                                xt[:, k * BC + c * CB : k * BC + (c + 1) * CB],
                                start=(k == 0),
                                stop=(k == 4),
                            )
                    x1p = xp.tile([128, 2 * CB], BF16, tag=f"x1p{c}")
                    for m in range(2):
                        nc.scalar.activation(
                            x1p[:, m * CB : (m + 1) * CB],
                            ps_x1[:, m * CB : (m + 1) * CB],
                            AF.Prelu,
                            bias=pp(CV_T1 + m),
                            scale=pp(CV_S1 + m),
                            alpha=pp(CV_A1),
                        )
                    gi = gip.tile([128, 6 * CB], BF16, tag=f"gi{c}")
                    gi_tiles.append(gi)
                    for m in range(6):
                        ps_gi = ps1.tile([128, CB], F32, tag="p")
                        for k in range(2):
                            nc.tensor.matmul(
                                ps_gi[:],
                                wih_t[:, k * 3 * H + m * 128 : k * 3 * H + (m + 1) * 128],
                                x1p[:, k * CB : (k + 1) * CB],
                                start=(k == 0),
                                stop=(k == 1),
                            )
                        # gi' = gi + (b_ih [+ b_hh for r,z]) folded via bias
                        nc.scalar.activation(
                            gi[:, m * CB : (m + 1) * CB],
                            ps_gi[:],
                            AF.Identity,
                            bias=pp(CV_GIB + m),
                        )

            # ---- GRU + stages 2..4; chunks interleaved per step so their
            # independent dependency chains overlap across engines ----
            hs = []
            h8s = []
            packs = [None] * NCH
            gin_bf = [gi_tiles[c][:, 4 * CB : 6 * CB] for c in range(NCH)]
            for c in range(NCH):
                h = hp.tile([128, 2 * CB], BF16, tag=f"h{c}")
                nc.vector.memset(h[:], 0.0)
                hs.append(h)
                h8 = hp8.tile([128, 2 * CB], FP8, tag=f"h8{c}")
                nc.vector.memset(h8[:], 0.0)
                h8s.append(h8)
            w2s = [None] * NCH
            y2s = [None] * NCH

            def emit_conv(t, c):
                """conv matmuls for step t (emitted at period t+1: inputs ready)"""
                ps_cv = ps2.tile([128, 2 * CB], F32, tag="g", name=f"pscv{c}")
                w2 = w2s[c]
                for m in range(2):
                    for k in range(2):
                        nc.tensor.matmul(
                            ps_cv[:, m * CB : (m + 1) * CB],
                            wc_t[:, k * H + m * 128 : k * H + (m + 1) * 128],
                            w2[:, k * CB : (k + 1) * CB],
                            start=(k == 0),
                            stop=False,
                        )
                    nc.tensor.matmul(
                        ps_cv[:, m * CB : (m + 1) * CB],
                        aux[:, m * 128 : (m + 1) * 128],
                        aux[:, H : H + CB],
                        start=False,
                        stop=True,
                    )
                return ps_cv

            def emit_prelu3(ps_cv, c):
                y2 = gp.tile([128, 2 * CB], BF16, tag="y2", name=f"y2_{c}")
                nc.scalar.activation(y2[:], ps_cv[:], AF.Prelu, alpha=pp(CV_A3))
                y2s[c] = y2

            def emit_wmu(t, c):
                """W_mu head for step t (emitted at period t+2), col-tiled so 4
                consecutive steps pack one PSUM bank [4l x 32c, b]"""
                j = t % 4
                if j == 0:
                    packs[c] = ps1.tile([128, CB], F32, tag="p", name=f"pack{c}")
                ps_pack = packs[c]
                y2 = y2s[c]
                for k in range(2):
                    nc.tensor.matmul(
                        ps_pack[32 * j : 32 * (j + 1), :],
                        wmu_t[:, k * C : (k + 1) * C],
                        y2[:, k * CB : (k + 1) * CB],
                        start=(k == 0),
                        stop=(k == 1),
                        tile_position=(0, 32 * j),
                    )

            def emit_pack_out(t, c):
                if t % 4 != 3:
                    return
                ps_pack = packs[c]
                pk = sp.tile([128, CB], F32, tag="pk", name=f"pk{c}")
                nc.scalar.activation(pk[:], ps_pack[:], AF.Identity, bias=pp(CV_BMU))
                tr = sp.tile([128, CB], F32, tag="tr", name=f"tr{c}")
                nc.vector.transpose(tr[:], pk[:])
                l4 = t // 4
                dst = out_h[:].rearrange(
                    "(c bh bl) (l4 li cc) -> c l4 li bl bh cc",
                    c=NCH, bh=CB // 32, bl=32, l4=L // 4, li=4, cc=C,
                )
                for li in range(4):
                    nc.sync.dma_start(
                        dst[c, l4, li],
                        tr[32 * li : 32 * (li + 1), :].rearrange(
                            "p (bh cc) -> p bh cc", bh=CB // 32
                        ),
                    )

            def emit_gate_mms_chunk(c, psr, psz, psn):
                # identity preloads put gi_r/gi_z into PSUM, then one fp8
                # DoubleRow matmul per (gate, half) accumulates the full K=256
                # W_hh contribution
                h8v = h8s[c][:].rearrange("p (k n) -> p k n", k=2)
                for g, ps in ((0, psr), (1, psz)):
                    for m in range(2):
                        nc.tensor.matmul(
                            ps[:, m * CB : (m + 1) * CB],
                            idtb,
                            gi_tiles[c][:, (2 * g + m) * CB : (2 * g + m + 1) * CB],
                            start=True,
                            stop=False,
                        )
                for g, ps in ((0, psr), (1, psz), (2, psn)):
                    for m in range(2):
                        nc.tensor.matmul(
                            ps[:, m * CB : (m + 1) * CB],
                            whh8(g, m),
                            h8v,
                            start=(g == 2),
                            stop=True,
                            perf_mode=DR,
                        )

            for t in range(L):
                psr, psz, psn = [], [], []
                for c in range(NCH):
                    psr.append(ps2.tile([128, 2 * CB], F32, tag="g", name=f"psr{c}"))
                    psz.append(ps2.tile([128, 2 * CB], F32, tag="g", name=f"psz{c}"))
                    psn.append(ps2.tile([128, 2 * CB], F32, tag="g", name=f"psn{c}"))
                for c in range(NCH):
                    emit_gate_mms_chunk(c, psr[c], psz[c], psn[c])
                # ready PE work fills the PE tail while the gate chains run
                pscs = [None] * NCH
                if t >= 1:
                    for c in range(NCH):
                        pscs[c] = emit_conv(t - 1, c)
                if t >= 2:
                    for c in range(NCH):
                        emit_wmu(t - 2, c)
                # per-chunk gate chains (staggered): chunk c's chain runs
                # while chunk c+1's matmuls are still on the PE
                for c in range(NCH):
                    r_sb = gp.tile([128, 2 * CB], BF16, tag="r", name=f"r{c}")
                    nc.scalar.activation(r_sb[:], psr[c][:], AF.Sigmoid)
                    z_sb = gp.tile([128, 2 * CB], BF16, tag="z", name=f"z{c}")
                    nc.scalar.activation(z_sb[:], psz[c][:], AF.Sigmoid)
                    t_sb = gp.tile([128, 2 * CB], BF16, tag="t", name=f"t{c}")
                    if BHN_ZERO:
                        nc.vector.tensor_tensor(t_sb[:], psn[c][:], r_sb[:], OP.mult)
                    else:
                        for m in range(2):
                            nc.vector.scalar_tensor_tensor(
                                t_sb[:, m * CB : (m + 1) * CB],
                                psn[c][:, m * CB : (m + 1) * CB],
                                pp(CV_BHN + m),
                                r_sb[:, m * CB : (m + 1) * CB],
                                op0=OP.add,
                                op1=OP.mult,
                            )
                    a_sb = gp.tile([128, 2 * CB], BF16, tag="a", name=f"a{c}")
                    nc.vector.tensor_tensor(a_sb[:], t_sb[:], gin_bf[c], OP.add)
                    n_sb = gp.tile([128, 2 * CB], BF16, tag="n", name=f"n{c}")
                    nc.scalar.activation(n_sb[:], a_sb[:], AF.Tanh)
                    u_sb = gp.tile([128, 2 * CB], BF16, tag="u", name=f"u{c}")
                    nc.vector.tensor_tensor(u_sb[:], hs[c][:], n_sb[:], OP.subtract)
                    v_sb = gp.tile([128, 2 * CB], BF16, tag="v", name=f"v{c}")
                    nc.vector.tensor_tensor(v_sb[:], z_sb[:], u_sb[:], OP.mult)
                    # h' = n + z*(h - n)
                    h = hp.tile([128, 2 * CB], BF16, tag=f"h{c}", name=f"h{c}")
                    nc.vector.tensor_tensor(h[:], n_sb[:], v_sb[:], OP.add)
                    hs[c] = h
                    if t < L - 1:
                        h8 = hp8.tile([128, 2 * CB], FP8, tag=f"h8{c}", name=f"h8{c}")
                        nc.gpsimd.tensor_copy(h8[:], h[:])
                        h8s[c] = h8
                    # fill ACT while the other chunk's matmuls finish
                    if t >= 1 and pscs[c] is not None:
                        emit_prelu3(pscs[c], c)
                # stage-2 prelu (off the recurrence): affine on DVE (4x TS),
                # alpha-mult + max on the idle GPSIMD engine
                for c in range(NCH):
                    w2a = gp.tile([128, 2 * CB], BF16, tag="w2a", name=f"w2a{c}")
                    for m in range(2):
                        nc.vector.tensor_scalar(
                            w2a[:, m * CB : (m + 1) * CB],
                            hs[c][:, m * CB : (m + 1) * CB],
                            pp(CV_S2 + m),
                            pp(CV_T2 + m),
                            op0=OP.mult,
                            op1=OP.add,
                        )
                    w2b = gp.tile([128, 2 * CB], BF16, tag="w2b", name=f"w2b{c}")
                    nc.vector.tensor_scalar_mul(w2b[:], w2a[:], pp(CV_A2))
                    w2 = gp.tile([128, 2 * CB], BF16, tag="w2", name=f"w2_{c}")
                    # prelu(x) = max(a*x, x) for 0 <= a <= 1
                    nc.vector.tensor_tensor(w2[:], w2a[:], w2b[:], OP.max)
                    w2s[c] = w2
                if t >= 2:
                    for c in range(NCH):
                        emit_pack_out(t - 2, c)
            # epilogue: drain the stage pipeline, chunk-interleaved
            pscs = [emit_conv(L - 1, c) for c in range(NCH)]
            for c in range(NCH):
                emit_wmu(L - 2, c)
            for c in range(NCH):
                emit_prelu3(pscs[c], c)
            for c in range(NCH):
                emit_pack_out(L - 2, c)
            for c in range(NCH):
                emit_wmu(L - 1, c)
            for c in range(NCH):
                emit_pack_out(L - 1, c)

    nc.compile()
    return nc


def _prep_inputs(inputs):
    f32 = np.float32
    x = np.ascontiguousarray(np.asarray(inputs["h_w_action"], f32).reshape(B, F))
    W_lin = np.asarray(inputs["W_lin"], f32)
    b_lin = np.asarray(inputs["b_lin"], f32)
    W_ih = np.asarray(inputs["W_ih"], f32)
    W_hh = np.asarray(inputs["W_hh"], f32)
    b_ih = np.asarray(inputs["b_ih"], f32)
    b_hh = np.asarray(inputs["b_hh"], f32)
    Wc = np.asarray(inputs["Wc"], f32)
    bc = np.asarray(inputs["bc"], f32)
    W_mu = np.asarray(inputs["W_mu"], f32)
    b_mu = np.asarray(inputs["b_mu"], f32)

    def bnfold(g, beta, m, v):
        s = g / np.sqrt(v + EPS)
        return s, beta - m * s

    s1, t1 = bnfold(inputs["g1"], inputs["beta1"], inputs["m1"], inputs["v1"])
    s2, t2 = bnfold(inputs["g2"], inputs["beta2"], inputs["m2"], inputs["v2"])
    s3, t3 = bnfold(inputs["g3"], inputs["beta3"], inputs["m3"], inputs["v3"])
    s1, t1, s2, t2, s3, t3 = (np.asarray(a, f32) for a in (s1, t1, s2, t2, s3, t3))
    t1 = t1 + s1 * b_lin          # fold linear bias into bn1 shift
    t3 = t3 + s3 * bc             # fold conv bias into bn3 shift
    gib = b_ih.copy()
    gib[: 2 * H] += b_hh[: 2 * H]  # fold b_hh into gi for the r,z gates
    bhn = b_hh[2 * H :]

    cv = np.zeros((128, NV), f32)
    for col, vec in ((CV_S1, s1), (CV_T1, t1), (CV_S2, s2), (CV_T2, t2),
                     (CV_S3, s3), (CV_T3, t3)):
        cv[:, col] = vec[:128]
        cv[:, col + 1] = vec[128:]
    for m in range(6):
        cv[:, CV_GIB + m] = gib[m * 128 : (m + 1) * 128]
    cv[:, CV_BHN] = bhn[:128]
    cv[:, CV_BHN + 1] = bhn[128:]
    cv[:, CV_BMU] = np.tile(b_mu, 4)
    cv[:, CV_A1] = f32(np.asarray(inputs["a1"]).reshape(-1)[0])
    cv[:, CV_A2] = f32(np.asarray(inputs["a2"]).reshape(-1)[0])
    cv[:, CV_A3] = f32(np.asarray(inputs["a3"]).reshape(-1)[0])

    bf = ml_dtypes.bfloat16
    f8 = ml_dtypes.float8_e4m3

    def kmaj(a, kt):  # [kt*128, n] -> [128, kt*n] k-tile-major columns
        n = a.shape[1]
        return a.reshape(kt, 128, n).transpose(1, 0, 2).reshape(128, kt * n)

    wp = np.concatenate(
        [
            kmaj(np.ascontiguousarray(W_lin.T), 5),
            kmaj(np.ascontiguousarray(W_ih.T), 2),
            kmaj(np.ascontiguousarray((Wc * s3[:, None]).T), 2),
            kmaj(np.ascontiguousarray(W_mu.T), 2),
            np.eye(128, dtype=f32),
        ],
        axis=1,
    ).astype(bf)
    # fp8 DoubleRow pack: [(gate, half) -> [128, k-tile(2), 128]]
    WhhT = np.ascontiguousarray(W_hh.T)       # [H(k), 3H]
    w8 = np.zeros((128, 1536), f8)
    for g in range(3):
        for m in range(2):
            for k in range(2):
                blk = WhhT[k * 128:(k + 1) * 128,
                           g * H + m * 128: g * H + (m + 1) * 128]
                col = (g * 2 + m) * 256 + k * 128
                w8[:, col:col + 128] = blk.astype(f8)
    shared = {
        "wp": np.ascontiguousarray(wp),
        "w8": np.ascontiguousarray(w8),
        "cv": cv,
        "aux": np.concatenate([t3, np.ones(CB, f32)]).reshape(1, -1).astype(bf),
    }
    in_maps = []
    for i in range(NCORES):
        m = dict(shared)
        xtc = np.ascontiguousarray(x[i * BC : (i + 1) * BC, :].T)  # [640, BC]
        m["xT"] = np.ascontiguousarray(kmaj(xtc, 5).astype(bf))
        in_maps.append(m)
    return in_maps


def kernel(**inputs) -> np.ndarray:
    bhn_zero = bool(np.all(np.asarray(inputs["b_hh"])[2 * H :] == 0))
    key = ("nc", bhn_zero)
    if key not in _CACHE:
        _CACHE[key] = build_program(bhn_zero)
    nc = _CACHE[key]
    _CACHE["last"] = nc
    in_maps = _prep_inputs(inputs)
    res = bass_utils.run_bass_kernel_spmd(nc, in_maps, core_ids=list(range(NCORES)))
    outs = [np.asarray(r["out"], np.float32) for r in res.results]
    return np.concatenate(outs, axis=0).reshape(E, S, L, C)


# revision 8
# speedup vs baseline: 2.9865x; 1.3691x over previous
"""Trainium2 Bass kernel for nn_Comm_OUT (Linear+BN+PReLU -> 20-step GRU ->
BN+PReLU -> 1x1 conv -> BN+PReLU -> linear head), data-parallel over 8 cores.

Layout strategy: everything on-chip is kept "transposed" (channels on SBUF
partitions, batch on the free dim) so the GRU recurrence never needs an
on-chip transpose:
    gh.T[3H, B] = W_hh @ h.T   (W_hh.T tiles are the stationary operand)
The recurrence matmuls run in fp8-e4m3 DoubleRow mode (K=256 per
instruction), per-step `gi + gh` adds are folded into the PE via identity-
matmul PSUM preloads, biases fold into per-partition activation operands,
and BN+PReLU collapses into single Prelu activations. The r/z sigmoids are
fused into one ACT instruction via a two-bank access pattern. The stage-2
prelu tail (w2b/max) runs on the otherwise idle GPSIMD engine.
"""

import numpy as np
import ml_dtypes

import concourse.bacc as bacc
import concourse.mybir as mybir
import concourse.tile as tile
from concourse import bass_utils

AF = mybir.ActivationFunctionType
OP = mybir.AluOpType
F32 = mybir.dt.float32
F32R = mybir.dt.float32r
BF16 = mybir.dt.bfloat16
FP8 = mybir.dt.float8e4
DR = mybir.MatmulPerfMode.DoubleRow

E, S, F, H, C, L = 64, 128, 640, 256, 32, 20
EPS = 1e-5
NCORES = 8
B = E * S              # 8192
BC = B // NCORES       # 1024 batch rows per core
NCH = 2                # chunks per core
CB = BC // NCH         # 512 batch rows per chunk (PSUM-bank friendly)

# const-vector column indices (packed [128, NV] tensor, one column per
# per-partition operand vector)
CV_S1, CV_T1, CV_GIB, CV_BHN, CV_S2, CV_T2, CV_S3, CV_T3 = 0, 2, 4, 10, 12, 14, 16, 18
CV_BMU, CV_A1, CV_A2, CV_A3 = 20, 21, 22, 23
NV = 24

# bf16 weight-pack column offsets (whh lives in a separate fp8 tensor)
WP_LIN, WP_IH, WP_C, WP_MU, WP_ID = 0, 1280, 2816, 3328, 3392
WP_COLS = 3520

_CACHE: dict = {}


def _mm(x):
    if x.dtype == F32:
        return x.bitcast(F32R)
    return x


def build_program(bhn_zero=True):
    BHN_ZERO = bhn_zero
    nc = bacc.Bacc("TRN2", target_bir_lowering=False, debug=False)

    xT_h = nc.dram_tensor("xT", [128, 5 * BC], BF16, kind="ExternalInput")
    wp_h = nc.dram_tensor("wp", [128, WP_COLS], BF16, kind="ExternalInput")
    w8_h = nc.dram_tensor("w8", [128, 1536], FP8, kind="ExternalInput")
    cv_h = nc.dram_tensor("cv", [128, NV], F32, kind="ExternalInput")
    aux_h = nc.dram_tensor("aux", [1, H + CB], BF16, kind="ExternalInput")
    out_h = nc.dram_tensor("out", [BC, L * C], F32, kind="ExternalOutput")

    with tile.TileContext(nc) as tc:
        with (
            tc.tile_pool(name="consts", bufs=1) as cpool,
            tc.tile_pool(name="gi", bufs=1) as gip,
            tc.tile_pool(name="hp", bufs=2) as hp,
            tc.tile_pool(name="hp8", bufs=2) as hp8,
            tc.tile_pool(name="gates", bufs=3) as gp,
            tc.tile_pool(name="s24", bufs=2) as sp,
            tc.tile_pool(name="ps2", bufs=3, space="PSUM") as ps2,
            tc.tile_pool(name="ps1", bufs=2, space="PSUM") as ps1,
        ):
            cvt = cpool.tile([128, NV], F32, tag="cv")
            nc.sync.dma_start(cvt[:], cv_h[:])
            wpt = cpool.tile([128, WP_COLS], BF16, tag="wpt")
            nc.sync.dma_start(wpt[:], wp_h[:])
            w8t = cpool.tile([128, 1536], FP8, tag="w8t")
            nc.sync.dma_start(w8t[:], w8_h[:])
            aux = cpool.tile([1, H + CB], BF16, tag="aux")
            nc.sync.dma_start(aux[:], aux_h[:])
            wlin_t = wpt[:, WP_LIN:WP_LIN + 1280]
            wih_t = wpt[:, WP_IH:WP_IH + 1536]
            wc_t = wpt[:, WP_C:WP_C + 512]
            wmu_t = wpt[:, WP_MU:WP_MU + 64]
            idtb = wpt[:, WP_ID:WP_ID + 128]

            def whh8(g, m):  # fp8 DR stationary [128, 2, 128] for gate g half m
                base = (g * 2 + m) * 256
                return w8t[:, base:base + 256].rearrange("p (k m) -> p k m", k=2)

            def pp(col):  # per-partition operand column
                return cvt[:, col : col + 1]

            # ---- stage 1: x1 = prelu(bn(x @ W_lin.T)), gi = x1 @ W_ih.T ----
            gi_tiles = []
            with tc.tile_pool(name="stage1", bufs=1) as xp:
                xt = xp.tile([128, 5 * BC], BF16, tag="xT")
                nc.sync.dma_start(xt[:], xT_h[:])
                for c in range(NCH):
                    ps_x1 = ps2.tile([128, 2 * CB], F32, tag="g")
                    for k in range(5):
                        for m in range(2):
                            nc.tensor.matmul(
                                ps_x1[:, m * CB : (m + 1) * CB],
                                wlin_t[:, k * H + m * 128 : k * H + (m + 1) * 128],
                                xt[:, k * BC + c * CB : k * BC + (c + 1) * CB],
                                start=(k == 0),
                                stop=(k == 4),
                            )
                    x1p = xp.tile([128, 2 * CB], BF16, tag=f"x1p{c}")
                    for m in range(2):
                        nc.scalar.activation(
                            x1p[:, m * CB : (m + 1) * CB],
                            ps_x1[:, m * CB : (m + 1) * CB],
                            AF.Prelu,
                            bias=pp(CV_T1 + m),
                            scale=pp(CV_S1 + m),
                            alpha=pp(CV_A1),
                        )
                    gi = gip.tile([128, 6 * CB], BF16, tag=f"gi{c}")
                    gi_tiles.append(gi)
                    for m in range(6):
                        ps_gi = ps1.tile([128, CB], F32, tag="p")
                        for k in range(2):
                            nc.tensor.matmul(
                                ps_gi[:],
                                wih_t[:, k * 3 * H + m * 128 : k * 3 * H + (m + 1) * 128],
                                x1p[:, k * CB : (k + 1) * CB],
                                start=(k == 0),
                                stop=(k == 1),
                            )
                        # gi' = gi + (b_ih [+ b_hh for r,z]) folded via bias
                        nc.scalar.activation(
                            gi[:, m * CB : (m + 1) * CB],
                            ps_gi[:],
                            AF.Identity,
                            bias=pp(CV_GIB + m),
                        )

            # ---- GRU + stages 2..4; chunks interleaved per step so their
            # independent dependency chains overlap across engines ----
            hs = []
            h8s = []
            packs = [None] * NCH
            gin_bf = [gi_tiles[c][:, 4 * CB : 6 * CB] for c in range(NCH)]
            for c in range(NCH):
                h = hp.tile([128, 2 * CB], BF16, tag=f"h{c}")
                nc.vector.memset(h[:], 0.0)
                hs.append(h)
                h8 = hp8.tile([128, 2 * CB], FP8, tag=f"h8{c}")
                nc.vector.memset(h8[:], 0.0)
                h8s.append(h8)
            w2s = [None] * NCH
            y2s = [None] * NCH

            def emit_conv(t, c):
                """conv matmuls for step t (emitted at period t+1: inputs ready)"""
                ps_cv = ps2.tile([128, 2 * CB], F32, tag="g", name=f"pscv{c}")
                w2 = w2s[c]
                for m in range(2):
                    for k in range(2):
                        nc.tensor.matmul(
                            ps_cv[:, m * CB : (m + 1) * CB],
                            wc_t[:, k * H + m * 128 : k * H + (m + 1) * 128],
                            w2[:, k * CB : (k + 1) * CB],
                            start=(k == 0),
                            stop=False,
                        )
                    nc.tensor.matmul(
                        ps_cv[:, m * CB : (m + 1) * CB],
                        aux[:, m * 128 : (m + 1) * 128],
                        aux[:, H : H + CB],
                        start=False,
                        stop=True,
                    )
                return ps_cv

            def emit_prelu3(ps_cv, c):
                y2 = gp.tile([128, 2 * CB], BF16, tag="y2", name=f"y2_{c}")
                nc.scalar.activation(y2[:], ps_cv[:], AF.Prelu, alpha=pp(CV_A3))
                y2s[c] = y2

            def emit_wmu(t, c):
                """W_mu head for step t (emitted at period t+2), col-tiled so 4
                consecutive steps pack one PSUM bank [4l x 32c, b]"""
                j = t % 4
                if j == 0:
                    packs[c] = ps1.tile([128, CB], F32, tag="p", name=f"pack{c}")
                ps_pack = packs[c]
                y2 = y2s[c]
                for k in range(2):
                    nc.tensor.matmul(
                        ps_pack[32 * j : 32 * (j + 1), :],
                        wmu_t[:, k * C : (k + 1) * C],
                        y2[:, k * CB : (k + 1) * CB],
                        start=(k == 0),
                        stop=(k == 1),
                        tile_position=(0, 32 * j),
                    )

            def emit_pack_out(t, c):
                if t % 4 != 3:
                    return
                ps_pack = packs[c]
                pk = sp.tile([128, CB], F32, tag="pk", name=f"pk{c}")
                nc.scalar.activation(pk[:], ps_pack[:], AF.Identity, bias=pp(CV_BMU))
                tr = sp.tile([128, CB], F32, tag="tr", name=f"tr{c}")
                nc.vector.transpose(tr[:], pk[:])
                l4 = t // 4
                dst = out_h[:].rearrange(
                    "(c bh bl) (l4 li cc) -> c l4 li bl bh cc",
                    c=NCH, bh=CB // 32, bl=32, l4=L // 4, li=4, cc=C,
                )
                for li in range(4):
                    nc.sync.dma_start(
                        dst[c, l4, li],
                        tr[32 * li : 32 * (li + 1), :].rearrange(
                            "p (bh cc) -> p bh cc", bh=CB // 32
                        ),
                    )

            def emit_gate_mms_chunk(c, psr, psz, psn):
                # identity preloads put gi_r/gi_z into PSUM, then one fp8
                # DoubleRow matmul per (gate, half) accumulates the full K=256
                # W_hh contribution
                h8v = h8s[c][:].rearrange("p (k n) -> p k n", k=2)
                for g, ps in ((0, psr), (1, psz)):
                    for m in range(2):
                        nc.tensor.matmul(
                            ps[:, m * CB : (m + 1) * CB],
                            idtb,
                            gi_tiles[c][:, (2 * g + m) * CB : (2 * g + m + 1) * CB],
                            start=True,
                            stop=False,
                        )
                for g, ps in ((0, psr), (1, psz), (2, psn)):
                    for m in range(2):
                        nc.tensor.matmul(
                            ps[:, m * CB : (m + 1) * CB],
                            whh8(g, m),
                            h8v,
                            start=(g == 2),
                            stop=True,
                            perf_mode=DR,
                        )

            for t in range(L):
                psr, psz, psn = [], [], []
                for c in range(NCH):
                    psr.append(ps2.tile([128, 2 * CB], F32, tag="g", name=f"psr{c}"))
                    psz.append(ps2.tile([128, 2 * CB], F32, tag="g", name=f"psz{c}"))
                    psn.append(ps2.tile([128, 2 * CB], F32, tag="g", name=f"psn{c}"))
                for c in range(NCH):
                    emit_gate_mms_chunk(c, psr[c], psz[c], psn[c])
                # ready PE work fills the PE tail while the gate chains run
                pscs = [None] * NCH
                if t >= 1:
                    for c in range(NCH):
                        pscs[c] = emit_conv(t - 1, c)
                if t >= 2:
                    for c in range(NCH):
                        emit_wmu(t - 2, c)
                # per-chunk gate chains (staggered): chunk c's chain runs
                # while chunk c+1's matmuls are still on the PE
                for c in range(NCH):
                    r_sb = gp.tile([128, 2 * CB], BF16, tag="r", name=f"r{c}")
                    nc.scalar.activation(r_sb[:], psr[c][:], AF.Sigmoid)
                    z_sb = gp.tile([128, 2 * CB], BF16, tag="z", name=f"z{c}")
                    nc.scalar.activation(z_sb[:], psz[c][:], AF.Sigmoid)
                    t_sb = gp.tile([128, 2 * CB], BF16, tag="t", name=f"t{c}")
                    if BHN_ZERO:
                        nc.vector.tensor_tensor(t_sb[:], psn[c][:], r_sb[:], OP.mult)
                    else:
                        for m in range(2):
                            nc.vector.scalar_tensor_tensor(
                                t_sb[:, m * CB : (m + 1) * CB],
                                psn[c][:, m * CB : (m + 1) * CB],
                                pp(CV_BHN + m),
                                r_sb[:, m * CB : (m + 1) * CB],
                                op0=OP.add,
                                op1=OP.mult,
                            )
                    a_sb = gp.tile([128, 2 * CB], BF16, tag="a", name=f"a{c}")
                    nc.vector.tensor_tensor(a_sb[:], t_sb[:], gin_bf[c], OP.add)
                    n_sb = gp.tile([128, 2 * CB], BF16, tag="n", name=f"n{c}")
                    nc.scalar.activation(n_sb[:], a_sb[:], AF.Tanh)
                    u_sb = gp.tile([128, 2 * CB], BF16, tag="u", name=f"u{c}")
                    nc.vector.tensor_tensor(u_sb[:], hs[c][:], n_sb[:], OP.subtract)
                    v_sb = gp.tile([128, 2 * CB], BF16, tag="v", name=f"v{c}")
                    nc.vector.tensor_tensor(v_sb[:], z_sb[:], u_sb[:], OP.mult)
                    # h' = n + z*(h - n)
                    h = hp.tile([128, 2 * CB], BF16, tag=f"h{c}", name=f"h{c}")
                    nc.vector.tensor_tensor(h[:], n_sb[:], v_sb[:], OP.add)
                    hs[c] = h
                    if t < L - 1:
                        h8 = hp8.tile([128, 2 * CB], FP8, tag=f"h8{c}", name=f"h8{c}")
                        nc.scalar.copy(h8[:], h[:])
                        h8s[c] = h8
                    # fill ACT while the other chunk's matmuls finish
                    if t >= 1 and pscs[c] is not None:
                        emit_prelu3(pscs[c], c)
                # stage-2 prelu (off the recurrence): affine on DVE (4x TS),
                # alpha-mult + max on the idle GPSIMD engine
                for c in range(NCH):
                    w2a = gp.tile([128, 2 * CB], BF16, tag="w2a", name=f"w2a{c}")
                    for m in range(2):
                        nc.vector.tensor_scalar(
                            w2a[:, m * CB : (m + 1) * CB],
                            hs[c][:, m * CB : (m + 1) * CB],
                            pp(CV_S2 + m),
                            pp(CV_T2 + m),
                            op0=OP.mult,
                            op1=OP.add,
                        )
                    w2b = gp.tile([128, 2 * CB], BF16, tag="w2b", name=f"w2b{c}")
                    nc.vector.tensor_scalar_mul(w2b[:], w2a[:], pp(CV_A2))
                    w2 = gp.tile([128, 2 * CB], BF16, tag="w2", name=f"w2_{c}")
                    # prelu(x) = max(a*x, x) for 0 <= a <= 1
                    nc.vector.tensor_tensor(w2[:], w2a[:], w2b[:], OP.max)
                    w2s[c] = w2
                if t >= 2:
                    for c in range(NCH):
                        emit_pack_out(t - 2, c)
            # epilogue: drain the stage pipeline, chunk-interleaved
            pscs = [emit_conv(L - 1, c) for c in range(NCH)]
            for c in range(NCH):
                emit_wmu(L - 2, c)
            for c in range(NCH):
                emit_prelu3(pscs[c], c)
            for c in range(NCH):
                emit_pack_out(L - 2, c)
            for c in range(NCH):
                emit_wmu(L - 1, c)
            for c in range(NCH):
                emit_pack_out(L - 1, c)

    nc.compile()
    return nc


def _prep_inputs(inputs):
    f32 = np.float32
    x = np.ascontiguousarray(np.asarray(inputs["h_w_action"], f32).reshape(B, F))
    W_lin = np.asarray(inputs["W_lin"], f32)
    b_lin = np.asarray(inputs["b_lin"], f32)
    W_ih = np.asarray(inputs["W_ih"], f32)
    W_hh = np.asarray(inputs["W_hh"], f32)
    b_ih = np.asarray(inputs["b_ih"], f32)
    b_hh = np.asarray(inputs["b_hh"], f32)
    Wc = np.asarray(inputs["Wc"], f32)
    bc = np.asarray(inputs["bc"], f32)
    W_mu = np.asarray(inputs["W_mu"], f32)
    b_mu = np.asarray(inputs["b_mu"], f32)

    def bnfold(g, beta, m, v):
        s = g / np.sqrt(v + EPS)
        return s, beta - m * s

    s1, t1 = bnfold(inputs["g1"], inputs["beta1"], inputs["m1"], inputs["v1"])
    s2, t2 = bnfold(inputs["g2"], inputs["beta2"], inputs["m2"], inputs["v2"])
    s3, t3 = bnfold(inputs["g3"], inputs["beta3"], inputs["m3"], inputs["v3"])
    s1, t1, s2, t2, s3, t3 = (np.asarray(a, f32) for a in (s1, t1, s2, t2, s3, t3))
    t1 = t1 + s1 * b_lin          # fold linear bias into bn1 shift
    t3 = t3 + s3 * bc             # fold conv bias into bn3 shift
    gib = b_ih.copy()
    gib[: 2 * H] += b_hh[: 2 * H]  # fold b_hh into gi for the r,z gates
    bhn = b_hh[2 * H :]

    cv = np.zeros((128, NV), f32)
    for col, vec in ((CV_S1, s1), (CV_T1, t1), (CV_S2, s2), (CV_T2, t2),
                     (CV_S3, s3), (CV_T3, t3)):
        cv[:, col] = vec[:128]
        cv[:, col + 1] = vec[128:]
    for m in range(6):
        cv[:, CV_GIB + m] = gib[m * 128 : (m + 1) * 128]
    cv[:, CV_BHN] = bhn[:128]
    cv[:, CV_BHN + 1] = bhn[128:]
    cv[:, CV_BMU] = np.tile(b_mu, 4)
    cv[:, CV_A1] = f32(np.asarray(inputs["a1"]).reshape(-1)[0])
    cv[:, CV_A2] = f32(np.asarray(inputs["a2"]).reshape(-1)[0])
    cv[:, CV_A3] = f32(np.asarray(inputs["a3"]).reshape(-1)[0])

    bf = ml_dtypes.bfloat16
    f8 = ml_dtypes.float8_e4m3

    def kmaj(a, kt):  # [kt*128, n] -> [128, kt*n] k-tile-major columns
        n = a.shape[1]
        return a.reshape(kt, 128, n).transpose(1, 0, 2).reshape(128, kt * n)

    wp = np.concatenate(
        [
            kmaj(np.ascontiguousarray(W_lin.T), 5),
            kmaj(np.ascontiguousarray(W_ih.T), 2),
            kmaj(np.ascontiguousarray((Wc * s3[:, None]).T), 2),
            kmaj(np.ascontiguousarray(W_mu.T), 2),
            np.eye(128, dtype=f32),
        ],
        axis=1,
    ).astype(bf)
    # fp8 DoubleRow pack: [(gate, half) -> [128, k-tile(2), 128]]
    WhhT = np.ascontiguousarray(W_hh.T)       # [H(k), 3H]
    w8 = np.zeros((128, 1536), f8)
    for g in range(3):
        for m in range(2):
            for k in range(2):
                blk = WhhT[k * 128:(k + 1) * 128,
                           g * H + m * 128: g * H + (m + 1) * 128]
                col = (g * 2 + m) * 256 + k * 128
                w8[:, col:col + 128] = blk.astype(f8)
    shared = {
        "wp": np.ascontiguousarray(wp),
        "w8": np.ascontiguousarray(w8),
        "cv": cv,
        "aux": np.concatenate([t3, np.ones(CB, f32)]).reshape(1, -1).astype(bf),
    }
    in_maps = []
    for i in range(NCORES):
        m = dict(shared)
        xtc = np.ascontiguousarray(x[i * BC : (i + 1) * BC, :].T)  # [640, BC]
        m["xT"] = np.ascontiguousarray(kmaj(xtc, 5).astype(bf))
        in_maps.append(m)
    return in_maps


def kernel(**inputs) -> np.ndarray:
    bhn_zero = bool(np.all(np.asarray(inputs["b_hh"])[2 * H :] == 0))
    key = ("nc", bhn_zero)
    if key not in _CACHE:
        _CACHE[key] = build_program(bhn_zero)
    nc = _CACHE[key]
    _CACHE["last"] = nc
    in_maps = _prep_inputs(inputs)
    res = bass_utils.run_bass_kernel_spmd(nc, in_maps, core_ids=list(range(NCORES)))
    outs = [np.asarray(r["out"], np.float32) for r in res.results]
    return np.concatenate(outs, axis=0).reshape(E, S, L, C)
